# revision 1
# baseline (speedup 1.0000x reference)
"""Trainium2 Bass kernel for 7x7 sliding-window self-similarity attention.

out[b,c,h,w] = sum_j softmax_j(x[h,w] * x[h+dh,w+dw]) * x[h+dh,w+dw]
over the 7x7 neighborhood (zero padding, pad=3).

Sharding: B*C = 256 independent 128x128 images, 32 images per core on 8
NeuronCores (pure data parallel, no collectives).

Per-core layout: partition p = rowblock(0..3)*32 + image(0..31); each
partition holds a 44-row x 140-col zero-padded fp32 slab (6160 contiguous
floats), so every 7x7 shift is a flat offset view. Elementwise ops run on
fully contiguous 1D runs spanning the pad columns (finite garbage there,
never read).

Score symmetry: e_{-d}[i] == e_d[i-d]; only 25 canonical score tiles are
computed on an extended halo run; mirrored contributions are views.

Numerator trick: sum_d e_d[i]*x[i+d] = (sum of t_d = e_d*s_d views)/x[i]
(s_d is the score itself), so both the +d and -d numerator contributions
are views of one t tile; the final division by x cancels exactly:
out = acc_t / (x * sum_e).

Engines: DVE does score/t products and the acc_t chain; ACT does exp;
TensorE accumulates sum_e into PSUM via fp32 (LOW_HIGH) identity
matmuls on its own SBUF ports; GpSimd stays idle (it shares DVE's
second SBUF read port - concurrency measured 3x slower on both).
"""

import numpy as np

import concourse.bacc as bacc
import concourse.bass as bass  # noqa: F401
import concourse.tile as tile
from concourse import mybir
from concourse.bass_utils import run_bass_kernel_spmd

N_CORES = 8
F32 = mybir.dt.float32
MULT = mybir.AluOpType.mult
ADD = mybir.AluOpType.add

B, C, H, W = 4, 64, 128, 128
N_IMG_TOTAL = B * C
IMG_PER_CORE = N_IMG_TOTAL // N_CORES  # 32
RB_N = 4
PAD = 6
MM_CHUNK = 512                # one PSUM bank of fp32


def canonical_offsets():
    canon = [(0, dj) for dj in range(1, 4)]
    canon += [(di, dj) for di in range(1, 4) for dj in range(-3, 4)]
    canon += [(0, 0)]
    return canon


def view2d(ap, off, rows, cols, stride):
    """Strided [rows, cols] view at element offset `off` of a flat [P, L] AP."""
    a = ap.copy()
    pair_t = type(a.ap)
    part = list(a.ap)[0]
    a.ap = pair_t([list(part), [stride, rows], [1, cols]])
    a.offset = a.offset + off
    return a


def build_nc(n_img=IMG_PER_CORE, h=H, w=W):
    br = h // RB_N               # 32
    wp = w + 2 * PAD             # 140
    slab = br + 2 * PAD          # 44
    P = n_img * RB_N             # 128

    nx = slab * wp               # 6160
    le = (br + 6) * wp + 8       # 5328 extended run
    soff = 3 * wp - 4
    la = br * wp                 # 4480 full-width run
    lc = br * w                  # 4096 compact output
    t0_off = 3 * wp + 4
    xq_off = 6 * wp
    mm_chunk = min(MM_CHUNK, lc)
    n_chunks = lc // mm_chunk
    rpc = mm_chunk // w

    nc = bacc.Bacc("TRN2", target_bir_lowering=False, debug=False)
    x_in = nc.dram_tensor("x", [P, nx], F32, kind="ExternalInput")
    id_in = nc.dram_tensor("ident", [P, P], F32, kind="ExternalInput")
    y_out = nc.dram_tensor("y", [P, lc], F32, kind="ExternalOutput")

    canon = canonical_offsets()
    n_views = 2 * len(canon) - 1  # 49

    with tile.TileContext(nc) as tc:
        with (
            tc.tile_pool(name="big", bufs=1) as big,
            tc.tile_pool(name="sp", bufs=2) as spool,
            tc.tile_pool(name="ep", bufs=2) as epool,
            tc.tile_pool(name="tp", bufs=1) as tpool,
            tc.tile_pool(name="fin", bufs=1) as fin,
            tc.tile_pool(name="ps", bufs=1, space="PSUM") as ps,
        ):
            x = big.tile([P, nx], F32, tag="x")
            ident = big.tile([P, P], F32, tag="id")
            acc = big.tile([P, la], F32, tag="acc")
            psum = ps.tile([P, lc], F32, tag="sum")

            # ranges actually read: [soff+1, soff+la+2*(3*wp+3)+1)
            rd_lo = soff + 1                       # 417
            p1_lo = soff + t0_off - 3              # first (0,dj) ops' start
            p1_hi = soff + t0_off + la + 3 + 1     # their end
            rd_hi = soff + t0_off + la + 3 * wp + 3 + 1  # global end (5744)
            nc.sync.dma_start(out=x[:, p1_lo:p1_hi],
                              in_=x_in[:, p1_lo:p1_hi])
            nc.sync.dma_start(out=x[:, rd_lo:p1_lo], in_=x_in[:, rd_lo:p1_lo])
            nc.sync.dma_start(out=x[:, p1_hi:rd_hi], in_=x_in[:, p1_hi:rd_hi])
            nc.sync.dma_start(out=ident[:], in_=id_in[:])

            vidx = 0
            aidx = 0

            def emit_score(di, dj):
                # s_d = x * shift(x, d) on the minimal run [t0-df, t0+la)
                df = di * wp + dj
                lo = t0_off - df
                ln = la + df
                s = spool.tile([P, le], F32, tag="s")
                e = epool.tile([P, le], F32, tag="e")
                sv = s[:, lo:lo + ln]
                if df == 0:
                    nc.scalar.activation(
                        out=sv, in_=x[:, soff + lo:soff + lo + ln],
                        func=mybir.ActivationFunctionType.Square,
                    )
                else:
                    nc.vector.tensor_tensor(
                        out=sv,
                        in0=x[:, soff + lo:soff + lo + ln],
                        in1=x[:, soff + lo + df:soff + lo + df + ln],
                        op=MULT,
                    )
                ev = e[:, lo:lo + ln]
                nc.scalar.activation(
                    out=ev, in_=sv, func=mybir.ActivationFunctionType.Exp
                )
                return s, e, sv, ev, df, lo, ln

            for k in range(len(canon)):
                s, e, sv, ev, df, lo, ln = emit_score(*canon[k])

                t = tpool.tile([P, le], F32, tag="t")
                nc.vector.tensor_tensor(out=t[:, lo:lo + ln], in0=ev,
                                        in1=sv, op=MULT)

                offs = [t0_off]
                if df != 0:
                    offs.append(t0_off - df)
                for to in offs:
                    tv = t[:, to:to + la]
                    if aidx == 0:
                        nc.scalar.copy(acc[:], tv)
                    else:
                        nc.vector.tensor_tensor(out=acc[:], in0=acc[:],
                                                in1=tv, op=ADD)
                    aidx += 1

                for to in offs:
                    eo = to + PAD
                    for ci in range(n_chunks):
                        mv = view2d(e[:], eo + ci * rpc * wp, rpc, w, wp)
                        nc.tensor.matmul(
                            psum[:, ci * mm_chunk:(ci + 1) * mm_chunk],
                            ident[:], mv,
                            start=(vidx == 0), stop=(vidx == n_views - 1),
                        )
                    vidx += 1

            half = lc // 2
            rows_h = half // w
            for hh in range(2):
                o = hh * half
                ro = hh * rows_h * wp
                den_h = fin.tile([P, half], F32, tag="den%d" % hh)
                r_h = fin.tile([P, half], F32, tag="r%d" % hh)
                xc = view2d(x[:], xq_off + PAD + ro, rows_h, w, wp)
                nc.vector.tensor_tensor(out=den_h[:], in0=psum[:, o:o + half],
                                        in1=xc, op=MULT)
                nc.vector.reciprocal_approx_fast(out=r_h[:], in_=den_h[:])
                out_h = fin.tile([P, half], F32, tag="den%d" % hh)
                av = view2d(acc[:], PAD + ro, rows_h, w, wp)
                nc.vector.tensor_tensor(out=out_h[:], in0=av, in1=r_h[:],
                                        op=MULT)
                nc.sync.dma_start(out=y_out[:, o:o + half], in_=out_h[:])
    nc.compile()
    return nc


_NC_CACHE = {}


def _get_nc():
    if "nc" not in _NC_CACHE:
        _NC_CACHE["nc"] = build_nc()
    return _NC_CACHE["nc"]


def make_slabs(imgs, h=H, w=W):
    """[n,h,w] fp32 -> [n*4, 44*140] slab layout (p = rb*n + img)."""
    n = imgs.shape[0]
    br = h // RB_N
    slab = br + 2 * PAD
    xp = np.pad(imgs, ((0, 0), (PAD, PAD), (PAD, PAD)))
    rows = (np.arange(RB_N) * br)[:, None] + np.arange(slab)
    sl = xp[:, rows, :]
    sl = sl.transpose(1, 0, 2, 3)
    return np.ascontiguousarray(sl.reshape(RB_N * n, -1))


def unslab_out(y, n_img, h=H, w=W):
    """[n*4, br*w compact] -> [n, h, w]."""
    br = h // RB_N
    y = y.reshape(RB_N, n_img, br, w).transpose(1, 0, 2, 3)
    return np.ascontiguousarray(y.reshape(n_img, h, w))


def run(x, **spmd_kwargs):
    nc = _get_nc()
    imgs = np.ascontiguousarray(np.asarray(x).reshape(N_IMG_TOTAL, H, W))
    imgs = imgs.astype(np.float32, copy=False)
    ident = np.eye(128, dtype=np.float32)
    in_maps = [
        {"x": make_slabs(imgs[i * IMG_PER_CORE:(i + 1) * IMG_PER_CORE]),
         "ident": ident}
        for i in range(N_CORES)
    ]
    res = run_bass_kernel_spmd(nc, in_maps, core_ids=list(range(N_CORES)),
                               **spmd_kwargs)
    out = np.concatenate(
        [unslab_out(res.results[i]["y"], IMG_PER_CORE) for i in range(N_CORES)],
        axis=0,
    )
    return out.reshape(B, C, H, W).astype(np.float32, copy=False), res


def kernel(x):
    out, _ = run(x)
    return out



# revision 2
# speedup vs baseline: 2.4425x; 2.4425x over previous
"""Trainium2 Bass kernel for 7x7 sliding-window self-similarity attention.

out[b,c,h,w] = sum_j softmax_j(x[h,w] * x[h+dh,w+dw]) * x[h+dh,w+dw]
over the 7x7 neighborhood (zero padding, pad=3).

Sharding: B*C = 256 independent 128x128 images, 32 images per core on 8
NeuronCores (pure data parallel, no collectives).

Per-core: TWO batches of 16 images x 8 rowblocks of 16 rows; partition
p = rowblock(0..7)*16 + image(0..15); each partition holds a 28-row x
140-col zero-padded bf16 slab (3920 contiguous elems), so every 7x7
shift is a flat offset view. Elementwise ops run on contiguous 1D runs
spanning the pad columns (finite garbage there, never read).

Score symmetry: e_{-d}[i] == e_d[i-d]; only 25 canonical score tiles are
computed on an extended halo run; mirrored contributions are views.

Numerator trick: sum_d e_d[i]*x[i+d] = (sum of t_d = e_d*s_d views)/x[i]
(s_d is the score itself), so both the +d and -d numerator contributions
are views of one t tile; the final division by x cancels exactly:
out = acc_t / (x * sum_e).

Precision: everything bf16 except the PSUM accumulators and the final
division (validated l2 ~ 3e-3 vs the 2e-2 gate). bf16 makes DVE
tensor_tensor run in 2x packed mode and matmuls 1 cycle/row (fp32 is 4).

Engines: DVE does score and t=e*s products (bf16 2x) plus the final
division; ACT does exp (bf16 out); TensorE accumulates BOTH sum_e and
acc_t into PSUM via bf16 identity matmuls. The two-batch split is what
lets both accumulators (2 x 2048 fp32/partition) fit PSUM's 8 banks.
"""

import numpy as np
import ml_dtypes

import concourse.bacc as bacc
import concourse.bass as bass  # noqa: F401
import concourse.tile as tile
from concourse import mybir
from concourse.bass_utils import run_bass_kernel_spmd

N_CORES = 8
F32 = mybir.dt.float32
BF16 = mybir.dt.bfloat16
NP_BF16 = ml_dtypes.bfloat16
MULT = mybir.AluOpType.mult
ADD = mybir.AluOpType.add

B, C, H, W = 4, 64, 128, 128
N_IMG_TOTAL = B * C
IMG_PER_CORE = N_IMG_TOTAL // N_CORES  # 32
N_BATCH = 2
IMG_PER_BATCH = IMG_PER_CORE // N_BATCH  # 16
RB_N = 8
BR = H // RB_N                 # 16 rows per rowblock
PAD = 6
WP = W + 2 * PAD               # 140
SLAB = BR + 2 * PAD            # 28
NX = SLAB * WP                 # 3920 bf16 elems per partition per batch
P = IMG_PER_BATCH * RB_N       # 128 partitions
LA = BR * WP                   # 2240 full-width run
LC = BR * W                    # 2048 compact output per batch
LE = (BR + 6) * WP + 8         # extended run tile size
T0 = 3 * WP + 4                # 424
SOFF = 3 * WP - 4              # 416
XQ = 6 * WP                    # 840 output base (row 6, col 0)
MM_CHUNK = 512                 # one PSUM bank of fp32
N_CHUNKS = LC // MM_CHUNK      # 4
RPC = MM_CHUNK // W            # 4 rows per chunk


def canonical_offsets():
    canon = [(0, dj) for dj in range(1, 4)]
    canon += [(di, dj) for di in range(1, 4) for dj in range(-3, 4)]
    canon += [(0, 0)]
    return canon


def view2d(ap, off, rows, cols, stride):
    """Strided [rows, cols] view at element offset `off` of a flat [P, L] AP."""
    a = ap.copy()
    pair_t = type(a.ap)
    part = list(a.ap)[0]
    a.ap = pair_t([list(part), [stride, rows], [1, cols]])
    a.offset = a.offset + off
    return a


def build_nc():
    nc = bacc.Bacc("TRN2", target_bir_lowering=False, debug=False)
    x_in = nc.dram_tensor("x", [P, N_BATCH * NX], BF16, kind="ExternalInput")
    id_in = nc.dram_tensor("ident", [P, P], BF16, kind="ExternalInput")
    y_out = nc.dram_tensor("y", [P, N_BATCH * LC], F32, kind="ExternalOutput")

    canon = canonical_offsets()
    n_views = 2 * len(canon) - 1  # 49

    # x ranges actually read, relative to a batch base
    rd_lo = SOFF + 1
    p1_lo = SOFF + T0 - 3
    p1_hi = SOFF + T0 + LA + 3 + 1
    rd_hi = SOFF + T0 + LA + 3 * WP + 3 + 1

    with tile.TileContext(nc) as tc:
        with (
            tc.tile_pool(name="big", bufs=1) as big,
            tc.tile_pool(name="sp", bufs=2) as spool,
            tc.tile_pool(name="ep", bufs=2) as epool,
            tc.tile_pool(name="tp", bufs=2) as tpool,
            tc.tile_pool(name="fin", bufs=2) as fin,
            tc.tile_pool(name="ps", bufs=1, space="PSUM") as ps,
        ):
            x = big.tile([P, N_BATCH * NX], BF16, tag="x")
            ident = big.tile([P, P], BF16, tag="id")

            # batch 0: the first score ops' range first so compute starts early
            nc.sync.dma_start(out=x[:, p1_lo:p1_hi], in_=x_in[:, p1_lo:p1_hi])
            nc.sync.dma_start(out=x[:, rd_lo:p1_lo], in_=x_in[:, rd_lo:p1_lo])
            nc.sync.dma_start(out=x[:, p1_hi:rd_hi], in_=x_in[:, p1_hi:rd_hi])
            nc.sync.dma_start(out=ident[:], in_=id_in[:])
            # batch 1 range streams under batch 0 compute
            nc.sync.dma_start(out=x[:, NX + rd_lo:NX + rd_hi],
                              in_=x_in[:, NX + rd_lo:NX + rd_hi])

            for b in range(N_BATCH):
                base = b * NX
                pe = ps.tile([P, LC], F32, tag="pe")
                pt = ps.tile([P, LC], F32, tag="pt")
                vidx = 0
                for k, (di, dj) in enumerate(canon):
                    df = di * WP + dj
                    lo = T0 - df
                    ln = LA + df
                    # even-align run start for bf16 2x packed mode
                    al = lo & 1
                    lo -= al
                    ln += al
                    s = spool.tile([P, LE], BF16, tag="s")
                    e = epool.tile([P, LE], BF16, tag="e")
                    t = tpool.tile([P, LE], BF16, tag="t")
                    sv = s[:, lo:lo + ln]
                    ev = e[:, lo:lo + ln]
                    tv = t[:, lo:lo + ln]
                    if df == 0:
                        nc.scalar.activation(
                            out=sv, in_=x[:, base + SOFF + lo:base + SOFF + lo + ln],
                            func=mybir.ActivationFunctionType.Square,
                        )
                    else:
                        nc.vector.tensor_tensor(
                            out=sv,
                            in0=x[:, base + SOFF + lo:base + SOFF + lo + ln],
                            in1=x[:, base + SOFF + lo + df:base + SOFF + lo + df + ln],
                            op=MULT,
                        )
                    nc.scalar.activation(
                        out=ev, in_=sv, func=mybir.ActivationFunctionType.Exp
                    )
                    nc.vector.tensor_tensor(out=tv, in0=ev, in1=sv, op=MULT)

                    offs = [T0]
                    if df != 0:
                        offs.append(T0 - df)
                    for to in offs:
                        eo = to + PAD
                        first = vidx == 0
                        last = vidx == n_views - 1
                        for ci in range(N_CHUNKS):
                            mve = view2d(e[:], eo + ci * RPC * WP, RPC, W, WP)
                            nc.tensor.matmul(
                                pe[:, ci * MM_CHUNK:(ci + 1) * MM_CHUNK],
                                ident[:], mve, start=first, stop=last,
                            )
                            mvt = view2d(t[:], eo + ci * RPC * WP, RPC, W, WP)
                            nc.tensor.matmul(
                                pt[:, ci * MM_CHUNK:(ci + 1) * MM_CHUNK],
                                ident[:], mvt, start=first, stop=last,
                            )
                        vidx += 1

                xc = view2d(x[:], base + XQ + PAD, BR, W, WP)
                den = fin.tile([P, LC], F32, tag="den")
                r = fin.tile([P, LC], F32, tag="r")
                out = fin.tile([P, LC], F32, tag="out")
                nc.vector.tensor_tensor(out=den[:], in0=pe[:], in1=xc, op=MULT)
                nc.vector.reciprocal_approx_fast(out=r[:], in_=den[:])
                nc.vector.tensor_tensor(out=out[:], in0=pt[:], in1=r[:], op=MULT)
                nc.sync.dma_start(out=y_out[:, b * LC:(b + 1) * LC], in_=out[:])
    nc.compile()
    return nc


_NC_CACHE = {}


def _get_nc():
    if "nc" not in _NC_CACHE:
        _NC_CACHE["nc"] = build_nc()
    return _NC_CACHE["nc"]


def make_slabs(imgs):
    """[32,128,128] fp32 (one core) -> [128, 2*3920] bf16 slab layout."""
    xb = imgs.astype(NP_BF16)
    xp = np.pad(xb, ((0, 0), (PAD, PAD), (PAD, PAD)))
    rows = (np.arange(RB_N) * BR)[:, None] + np.arange(SLAB)
    out = np.empty((P, N_BATCH, NX), NP_BF16)
    for b in range(N_BATCH):
        part = xp[b * IMG_PER_BATCH:(b + 1) * IMG_PER_BATCH]  # [16,140,140]
        sl = part[:, rows, :]              # [16, 8, 28, 140]
        sl = sl.transpose(1, 0, 2, 3)      # [8, 16, 28, 140] p = rb*16+img
        out[:, b, :] = sl.reshape(P, NX)
    return np.ascontiguousarray(out.reshape(P, N_BATCH * NX))


def unslab_out(y):
    """[128, 2*2048] fp32 -> [32, 128, 128]."""
    res = np.empty((IMG_PER_CORE, H, W), np.float32)
    for b in range(N_BATCH):
        yb = y[:, b * LC:(b + 1) * LC].reshape(RB_N, IMG_PER_BATCH, BR, W)
        res[b * IMG_PER_BATCH:(b + 1) * IMG_PER_BATCH] = (
            yb.transpose(1, 0, 2, 3).reshape(IMG_PER_BATCH, H, W)
        )
    return res


def run(x, **spmd_kwargs):
    nc = _get_nc()
    imgs = np.ascontiguousarray(np.asarray(x).reshape(N_IMG_TOTAL, H, W))
    imgs = imgs.astype(np.float32, copy=False)
    ident = np.eye(P, dtype=NP_BF16)
    in_maps = [
        {"x": make_slabs(imgs[i * IMG_PER_CORE:(i + 1) * IMG_PER_CORE]),
         "ident": ident}
        for i in range(N_CORES)
    ]
    res = run_bass_kernel_spmd(nc, in_maps, core_ids=list(range(N_CORES)),
                               **spmd_kwargs)
    out = np.concatenate(
        [unslab_out(res.results[i]["y"]) for i in range(N_CORES)],
        axis=0,
    )
    return out.reshape(B, C, H, W).astype(np.float32, copy=False), res


def kernel(x):
    out, _ = run(x)
    return out


# revision 5
# speedup vs baseline: 2.5696x; 1.0520x over previous
"""Trainium2 Bass kernel for 7x7 sliding-window self-similarity attention.

out[b,c,h,w] = sum_j softmax_j(x[h,w] * x[h+dh,w+dw]) * x[h+dh,w+dw]
over the 7x7 neighborhood (zero padding, pad=3).

Sharding: B*C = 256 independent 128x128 images, 32 images per core on 8
NeuronCores (pure data parallel, no collectives).

Per-core: TWO batches of 16 images x 8 rowblocks of 16 rows; partition
p = rowblock(0..7)*16 + image(0..15); each partition holds a 28-row x
136-col zero-padded bf16 slab (3808 contiguous elems; 6-row / 4-col
halo), so every 7x7 shift is a flat offset view. Elementwise ops run on
contiguous 1D runs spanning the pad columns (finite garbage there,
never read).

Score symmetry: e_{-d}[i] == e_d[i-d]; only 25 canonical score tiles are
computed on an extended halo run; mirrored contributions are views.

Numerator trick: sum_d e_d[i]*x[i+d] = (sum of t_d = e_d*s_d views)/x[i]
(s_d is the score itself), so both the +d and -d numerator contributions
are views of one t tile; the final division by x cancels exactly:
out = acc_t / (x * sum_e).

Precision: everything bf16 except the PSUM accumulators and the final
division (validated l2 ~ 3e-3 vs the 2e-2 gate). bf16 makes DVE
tensor_tensor run in 2x packed mode and matmuls 1 cycle/row (fp32 is 4).

Engines: DVE does score and t=e*s products (bf16 2x) plus the final
division; ACT does exp (bf16 out); TensorE accumulates BOTH sum_e and
acc_t into PSUM via bf16 identity matmuls (the PE array is the
bottleneck at ~0.23 ns/row, so the PREADD view-pairs are pre-added on
DVE to shave PE work). The two-batch split is what lets both
accumulators (2 x 2048 fp32/partition) fit PSUM's 8 banks; PSUM lives
as 4 half tiles so the final division releases banks to the next batch
early, and the first tile is computed in two half-runs so the PE starts
as soon as the first partial DMA lands.
"""

import numpy as np
import ml_dtypes

import concourse.bacc as bacc
import concourse.bass as bass  # noqa: F401
import concourse.tile as tile
from concourse import mybir
from concourse.bass_utils import run_bass_kernel_spmd

N_CORES = 8
F32 = mybir.dt.float32
BF16 = mybir.dt.bfloat16
NP_BF16 = ml_dtypes.bfloat16
MULT = mybir.AluOpType.mult
ADD = mybir.AluOpType.add

B, C, H, W = 4, 64, 128, 128
N_IMG_TOTAL = B * C
IMG_PER_CORE = N_IMG_TOTAL // N_CORES  # 32
N_BATCH = 2
IMG_PER_BATCH = IMG_PER_CORE // N_BATCH  # 16
RB_N = 8
BR = H // RB_N                 # 16 rows per rowblock
PADV = 6
PADH = 4
WP = W + 2 * PADH              # 136
SLAB = BR + 2 * PADV           # 28
NX = SLAB * WP                 # 3808 bf16 elems per partition per batch
P = IMG_PER_BATCH * RB_N       # 128 partitions
LA = BR * WP                   # 2176 full-width run
LC = BR * W                    # 2048 compact output per batch
T0 = PADV * WP + PADH          # 820: flat index of pixel (0,0)
LE = T0 + LA + 8               # tile size covering all runs
DF_MAX = 3 * WP + 3            # 411
MM_CHUNK = 512                 # one PSUM bank of fp32
N_CHUNKS = LC // MM_CHUNK      # 4
RPC = MM_CHUNK // W            # 4 rows per chunk
HALF = LC // 2                 # 1024 (two PSUM banks)

# canonical offsets, ordered: first tile warms the PE fast, the
# PREADD (di,0) tiles sit mid-stream, (0,0) (single view) is last
PREADD = [(1, 0), (2, 0)]
CANON = (
    [(3, 0)]
    + [(1, dj) for dj in (-3, -2, -1, 1, 2, 3)]
    + [(2, dj) for dj in (-3, -2, -1, 1, 2, 3)]
    + [(3, dj) for dj in (-3, -2, -1, 1, 2, 3)]
    + PREADD
    + [(0, 1), (0, 2), (0, 3)]
    + [(0, 0)]
)
DF0 = 3 * WP                   # tile 0 is (3,0)


def view2d(ap, off, rows, cols, stride):
    """Strided [rows, cols] view at element offset `off` of a flat [P, L] AP."""
    a = ap.copy()
    pair_t = type(a.ap)
    part = list(a.ap)[0]
    a.ap = pair_t([list(part), [stride, rows], [1, cols]])
    a.offset = a.offset + off
    return a


def build_nc():
    nc = bacc.Bacc("TRN2", target_bir_lowering=False, debug=False)
    x_in = nc.dram_tensor("x", [P, N_BATCH * NX], BF16, kind="ExternalInput")
    id_in = nc.dram_tensor("ident", [P, P], BF16, kind="ExternalInput")
    y_out = nc.dram_tensor("y", [P, N_BATCH * LC], F32, kind="ExternalOutput")

    rd_lo = T0 - DF_MAX - 1            # 408
    rd_hi = T0 + LA + DF_MAX + 1       # 3408
    ln0 = LA + DF0
    mid0 = (T0 - DF0) + (ln0 // 2 + 1) // 2 * 2   # even midpoint of run 0
    p1_hi = mid0 + DF0                 # in1 of tile 0's first half ends here

    with tile.TileContext(nc) as tc:
        with (
            tc.tile_pool(name="big", bufs=1) as big,
            tc.tile_pool(name="sp", bufs=3) as spool,
            tc.tile_pool(name="ep", bufs=3) as epool,
            tc.tile_pool(name="tp", bufs=3) as tpool,
            tc.tile_pool(name="up", bufs=2) as upool,
            tc.tile_pool(name="fin", bufs=2) as fin,
            tc.tile_pool(name="ps", bufs=1, space="PSUM") as ps,
        ):
            x = big.tile([P, N_BATCH * NX], BF16, tag="x")
            ident = big.tile([P, P], BF16, tag="id")

            # batch 0: first the range tile 0's first half-run needs
            nc.sync.dma_start(out=x[:, rd_lo:p1_hi], in_=x_in[:, rd_lo:p1_hi])
            nc.sync.dma_start(out=ident[:], in_=id_in[:])
            nc.sync.dma_start(out=x[:, p1_hi:rd_hi], in_=x_in[:, p1_hi:rd_hi])
            # batch 1 range streams under batch 0 compute
            nc.sync.dma_start(out=x[:, NX + rd_lo:NX + rd_hi],
                              in_=x_in[:, NX + rd_lo:NX + rd_hi])

            def emit_tile(b, k):
                base = b * NX
                di, dj = CANON[k]
                df = di * WP + dj
                lo = T0 - df
                ln = LA + df
                al = lo & 1
                lo -= al
                ln += al
                s = spool.tile([P, LE], BF16, tag="s")
                e = epool.tile([P, LE], BF16, tag="e")
                t = tpool.tile([P, LE], BF16, tag="t")
                # split the very first tile so the PE starts early
                if b == 0 and k == 0:
                    cuts = [lo, mid0, lo + ln]
                else:
                    cuts = [lo, lo + ln]
                for ci in range(len(cuts) - 1):
                    c0, c1 = cuts[ci], cuts[ci + 1]
                    sv = s[:, c0:c1]
                    ev = e[:, c0:c1]
                    tv = t[:, c0:c1]
                    if df == 0:
                        nc.scalar.activation(
                            out=sv, in_=x[:, base + c0:base + c1],
                            func=mybir.ActivationFunctionType.Square,
                        )
                    else:
                        nc.vector.tensor_tensor(
                            out=sv,
                            in0=x[:, base + c0:base + c1],
                            in1=x[:, base + c0 + df:base + c1 + df],
                            op=MULT,
                        )
                    nc.scalar.activation(
                        out=ev, in_=sv, func=mybir.ActivationFunctionType.Exp
                    )
                    nc.vector.tensor_tensor(out=tv, in0=ev, in1=sv, op=MULT)
                return s, e, t, df

            def emit_mms(k, tiles, pe, pt, first, chunks=range(N_CHUNKS)):
                s, e, t, df = tiles
                stop = k == len(CANON) - 1
                if CANON[k] in PREADD:
                    # collapse the two views into one via a DVE pre-add
                    ue = upool.tile([P, LA], BF16, tag="ue")
                    ut = upool.tile([P, LA], BF16, tag="ut")
                    nc.vector.tensor_tensor(
                        out=ue[:], in0=e[:, T0:T0 + LA],
                        in1=e[:, T0 - df:T0 - df + LA], op=ADD)
                    nc.vector.tensor_tensor(
                        out=ut[:], in0=t[:, T0:T0 + LA],
                        in1=t[:, T0 - df:T0 - df + LA], op=ADD)
                    views = [(ue, ut, 0)]
                else:
                    views = [(e, t, T0)]
                    if df != 0:
                        views.append((e, t, T0 - df))
                for esrc, tsrc, to in views:
                    for ci in chunks:
                        pev = pe[ci // 2][:, (ci % 2) * MM_CHUNK:
                                          (ci % 2 + 1) * MM_CHUNK]
                        ptv = pt[ci // 2][:, (ci % 2) * MM_CHUNK:
                                          (ci % 2 + 1) * MM_CHUNK]
                        mve = view2d(esrc[:], to + ci * RPC * WP, RPC, W, WP)
                        nc.tensor.matmul(pev, ident[:], mve,
                                         start=first[ci], stop=stop)
                        mvt = view2d(tsrc[:], to + ci * RPC * WP, RPC, W, WP)
                        nc.tensor.matmul(ptv, ident[:], mvt,
                                         start=first[ci], stop=stop)
                        first[ci] = False

            def emit_final_half(b, h, pe, pt):
                base = b * NX
                xc = view2d(x[:], base + T0 + h * (BR // 2) * WP,
                            BR // 2, W, WP)
                den = fin.tile([P, HALF], F32, tag="den%d" % h)
                r = fin.tile([P, HALF], F32, tag="r%d" % h)
                out = fin.tile([P, HALF], F32, tag="out%d" % h)
                nc.vector.tensor_tensor(out=den[:], in0=pe[h][:], in1=xc,
                                        op=MULT)
                nc.vector.reciprocal_approx_fast(out=r[:], in_=den[:])
                nc.vector.tensor_tensor(out=out[:], in0=pt[h][:], in1=r[:],
                                        op=MULT)
                nc.sync.dma_start(
                    out=y_out[:, b * LC + h * HALF:b * LC + (h + 1) * HALF],
                    in_=out[:])

            pe_prev = pt_prev = None
            for b in range(N_BATCH):
                pe = [ps.tile([P, HALF], F32, tag="pe%d" % i,
                              name="pe%d" % i) for i in (0, 1)]
                pt = [ps.tile([P, HALF], F32, tag="pt%d" % i,
                              name="pt%d" % i) for i in (0, 1)]
                first = [True] * N_CHUNKS
                if b == 0:
                    for k in range(len(CANON)):
                        emit_mms(k, emit_tile(b, k), pe, pt, first)
                else:
                    # overlap the previous batch's finals (which release the
                    # PSUM banks) with this batch's first tile computes
                    t0_tiles = emit_tile(b, 0)
                    emit_final_half(b - 1, 0, pe_prev, pt_prev)
                    emit_mms(0, t0_tiles, pe, pt, first, chunks=(0, 1))
                    t1_tiles = emit_tile(b, 1)
                    emit_final_half(b - 1, 1, pe_prev, pt_prev)
                    emit_mms(0, t0_tiles, pe, pt, first, chunks=(2, 3))
                    emit_mms(1, t1_tiles, pe, pt, first)
                    for k in range(2, len(CANON)):
                        emit_mms(k, emit_tile(b, k), pe, pt, first)
                pe_prev, pt_prev = pe, pt
            emit_final_half(N_BATCH - 1, 0, pe_prev, pt_prev)
            emit_final_half(N_BATCH - 1, 1, pe_prev, pt_prev)
    nc.compile()
    return nc


_NC_CACHE = {}


def _get_nc():
    if "nc" not in _NC_CACHE:
        _NC_CACHE["nc"] = build_nc()
    return _NC_CACHE["nc"]


def make_slabs(imgs):
    """[32,128,128] fp32 (one core) -> [128, 2*3808] bf16 slab layout."""
    xb = imgs.astype(NP_BF16)
    xp = np.pad(xb, ((0, 0), (PADV, PADV), (PADH, PADH)))
    rows = (np.arange(RB_N) * BR)[:, None] + np.arange(SLAB)
    out = np.empty((P, N_BATCH, NX), NP_BF16)
    for b in range(N_BATCH):
        part = xp[b * IMG_PER_BATCH:(b + 1) * IMG_PER_BATCH]  # [16,140,136]
        sl = part[:, rows, :]              # [16, 8, 28, 136]
        sl = sl.transpose(1, 0, 2, 3)      # [8, 16, 28, 136] p = rb*16+img
        out[:, b, :] = sl.reshape(P, NX)
    return np.ascontiguousarray(out.reshape(P, N_BATCH * NX))


def unslab_out(y):
    """[128, 2*2048] fp32 -> [32, 128, 128]."""
    res = np.empty((IMG_PER_CORE, H, W), np.float32)
    for b in range(N_BATCH):
        yb = y[:, b * LC:(b + 1) * LC].reshape(RB_N, IMG_PER_BATCH, BR, W)
        res[b * IMG_PER_BATCH:(b + 1) * IMG_PER_BATCH] = (
            yb.transpose(1, 0, 2, 3).reshape(IMG_PER_BATCH, H, W)
        )
    return res


def run(x, **spmd_kwargs):
    nc = _get_nc()
    imgs = np.ascontiguousarray(np.asarray(x).reshape(N_IMG_TOTAL, H, W))
    imgs = imgs.astype(np.float32, copy=False)
    ident = np.eye(P, dtype=NP_BF16)
    in_maps = [
        {"x": make_slabs(imgs[i * IMG_PER_CORE:(i + 1) * IMG_PER_CORE]),
         "ident": ident}
        for i in range(N_CORES)
    ]
    res = run_bass_kernel_spmd(nc, in_maps, core_ids=list(range(N_CORES)),
                               **spmd_kwargs)
    out = np.concatenate(
        [unslab_out(res.results[i]["y"]) for i in range(N_CORES)],
        axis=0,
    )
    return out.reshape(B, C, H, W).astype(np.float32, copy=False), res


def kernel(x):
    out, _ = run(x)
    return out


# revision 11
# speedup vs baseline: 2.5836x; 1.0055x over previous
"""Trainium2 Bass kernel for 7x7 sliding-window self-similarity attention.

out[b,c,h,w] = sum_j softmax_j(x[h,w] * x[h+dh,w+dw]) * x[h+dh,w+dw]
over the 7x7 neighborhood (zero padding, pad=3).

Sharding: B*C = 256 independent 128x128 images, 32 images per core on 8
NeuronCores (pure data parallel, no collectives).

Per-core: TWO batches of 16 images x 8 rowblocks of 16 rows; partition
p = rowblock(0..7)*16 + image(0..15); each partition holds a 28-row x
136-col zero-padded bf16 slab (3808 contiguous elems; 6-row / 4-col
halo), so every 7x7 shift is a flat offset view. Elementwise ops run on
contiguous 1D runs spanning the pad columns (finite garbage there,
never read).

Score symmetry: e_{-d}[i] == e_d[i-d]; only 25 canonical score tiles are
computed on an extended halo run; mirrored contributions are views.

Numerator trick: sum_d e_d[i]*x[i+d] = (sum of t_d = e_d*s_d views)/x[i]
(s_d is the score itself), so both the +d and -d numerator contributions
are views of one t tile; the final division by x cancels exactly:
out = acc_t / (x * sum_e).

Precision: everything bf16 except the PSUM accumulators and the final
division (validated l2 ~ 3e-3 vs the 2e-2 gate). bf16 makes DVE
tensor_tensor run in 2x packed mode and matmuls 1 cycle/row (fp32 is 4).

Engines: DVE does score and t=e*s products (bf16 2x) plus the final
division; ACT does exp (bf16 out); TensorE accumulates BOTH sum_e and
acc_t into PSUM via bf16 identity matmuls (the PE array is the
bottleneck at ~0.23 ns/row, so the PREADD view-pairs are pre-added on
DVE to shave PE work). The two-batch split is what lets both
accumulators (2 x 2048 fp32/partition) fit PSUM's 8 banks; PSUM lives
as 4 half tiles so the final division releases banks to the next batch
early, and the first tile is computed in two half-runs so the PE starts
as soon as the first partial DMA lands.
"""

import numpy as np
import ml_dtypes

import concourse.bacc as bacc
import concourse.bass as bass  # noqa: F401
import concourse.tile as tile
from concourse import mybir
from concourse.bass_utils import run_bass_kernel_spmd

N_CORES = 8
F32 = mybir.dt.float32
BF16 = mybir.dt.bfloat16
NP_BF16 = ml_dtypes.bfloat16
MULT = mybir.AluOpType.mult
ADD = mybir.AluOpType.add

B, C, H, W = 4, 64, 128, 128
N_IMG_TOTAL = B * C
IMG_PER_CORE = N_IMG_TOTAL // N_CORES  # 32
N_BATCH = 2
IMG_PER_BATCH = IMG_PER_CORE // N_BATCH  # 16
RB_N = 8
BR = H // RB_N                 # 16 rows per rowblock
PADV = 6
PADH = 4
WP = W + 2 * PADH              # 136
SLAB = BR + 2 * PADV           # 28
NX = SLAB * WP                 # 3808 bf16 elems per partition per batch
P = IMG_PER_BATCH * RB_N       # 128 partitions
LA = BR * WP                   # 2176 full-width run
LC = BR * W                    # 2048 compact output per batch
T0 = PADV * WP + PADH          # 820: flat index of pixel (0,0)
LE = T0 + LA + 8               # tile size covering all runs
DF_MAX = 3 * WP + 3            # 411
MM_CHUNK = 512                 # one PSUM bank of fp32
N_CHUNKS = LC // MM_CHUNK      # 4
RPC = MM_CHUNK // W            # 4 rows per chunk
HALF = LC // 2                 # 1024 (two PSUM banks)

# canonical offsets, ordered: first tile warms the PE fast, the
# PREADD tiles sit mid-stream, (0,0) (single view) is last; the last
# N_TAIL tiles emit all their e-matmuls before their t-matmuls so the
# final division can overlap the trailing t accumulation
PREADD = [(1, 0), (2, 0), (0, 2)]
CANON = (
    [(3, 0)]
    + [(1, dj) for dj in (-3, -2, -1, 1, 2, 3)]
    + [(2, dj) for dj in (-3, -2, -1, 1, 2, 3)]
    + [(3, dj) for dj in (-3, -2, -1, 1, 2, 3)]
    + PREADD
    + [(0, 1), (0, 3)]
    + [(0, 0)]
)
N_TAIL = 3                     # (0,1), (0,3), (0,0)
DF0 = 3 * WP                   # tile 0 is (3,0)


def view2d(ap, off, rows, cols, stride):
    """Strided [rows, cols] view at element offset `off` of a flat [P, L] AP."""
    a = ap.copy()
    pair_t = type(a.ap)
    part = list(a.ap)[0]
    a.ap = pair_t([list(part), [stride, rows], [1, cols]])
    a.offset = a.offset + off
    return a


def build_nc():
    nc = bacc.Bacc("TRN2", target_bir_lowering=False, debug=False)
    x_in = nc.dram_tensor("x", [P, N_BATCH * NX], BF16, kind="ExternalInput")
    id_in = nc.dram_tensor("ident", [P, P], BF16, kind="ExternalInput")
    y_out = nc.dram_tensor("y", [P, N_BATCH * LC], F32, kind="ExternalOutput")

    rd_lo = T0 - DF_MAX - 1            # 408
    rd_hi = T0 + LA + DF_MAX + 1       # 3408
    lo0 = T0 - DF0                     # 412
    ln0 = LA + DF0
    # tile 0 computed in three chunks so the PE starts on the first
    # partial DMA; chunk boundaries even-aligned
    cut1 = lo0 + 648                   # 1060
    cut2 = lo0 + ln0 // 2 + 2          # 1704
    cuts0 = [lo0, cut1, cut2, lo0 + ln0]

    with tile.TileContext(nc) as tc:
        with (
            tc.tile_pool(name="big", bufs=1) as big,
            tc.tile_pool(name="sp", bufs=3) as spool,
            tc.tile_pool(name="ep", bufs=3) as epool,
            tc.tile_pool(name="tp", bufs=3) as tpool,
            tc.tile_pool(name="up", bufs=2) as upool,
            tc.tile_pool(name="fin", bufs=2) as fin,
            tc.tile_pool(name="ps", bufs=1, space="PSUM") as ps,
        ):
            x = big.tile([P, N_BATCH * NX], BF16, tag="x")
            ident = big.tile([P, P], BF16, tag="id")

            # batch 0 input in 3 pieces matching tile 0's chunk needs
            d1 = cut1 + DF0
            d2 = cut2 + DF0
            nc.sync.dma_start(out=x[:, rd_lo:d1], in_=x_in[:, rd_lo:d1])
            nc.sync.dma_start(out=ident[:], in_=id_in[:])
            nc.sync.dma_start(out=x[:, d1:d2], in_=x_in[:, d1:d2])
            nc.sync.dma_start(out=x[:, d2:rd_hi], in_=x_in[:, d2:rd_hi])
            # batch 1 range streams under batch 0 compute
            nc.sync.dma_start(out=x[:, NX + rd_lo:NX + rd_hi],
                              in_=x_in[:, NX + rd_lo:NX + rd_hi])

            def emit_tile(b, k):
                base = b * NX
                di, dj = CANON[k]
                df = di * WP + dj
                lo = T0 - df
                ln = LA + df
                al = lo & 1
                lo -= al
                ln += al
                s = spool.tile([P, LE], BF16, tag="s")
                e = epool.tile([P, LE], BF16, tag="e")
                t = tpool.tile([P, LE], BF16, tag="t")
                # split the very first tile so the PE starts early
                if b == 0 and k == 0:
                    cuts = cuts0
                else:
                    cuts = [lo, lo + ln]
                for ci in range(len(cuts) - 1):
                    c0, c1 = cuts[ci], cuts[ci + 1]
                    sv = s[:, c0:c1]
                    ev = e[:, c0:c1]
                    tv = t[:, c0:c1]
                    if df == 0:
                        nc.scalar.activation(
                            out=sv, in_=x[:, base + c0:base + c1],
                            func=mybir.ActivationFunctionType.Square,
                        )
                    else:
                        nc.vector.tensor_tensor(
                            out=sv,
                            in0=x[:, base + c0:base + c1],
                            in1=x[:, base + c0 + df:base + c1 + df],
                            op=MULT,
                        )
                    nc.scalar.activation(
                        out=ev, in_=sv, func=mybir.ActivationFunctionType.Exp
                    )
                    nc.vector.tensor_tensor(out=tv, in0=ev, in1=sv, op=MULT)
                return s, e, t, df

            def emit_preadd(tiles):
                s, e, t, df = tiles
                # collapse the two views into one via a DVE pre-add
                ue = upool.tile([P, LA], BF16, tag="ue")
                ut = upool.tile([P, LA], BF16, tag="ut")
                nc.vector.tensor_tensor(
                    out=ue[:], in0=e[:, T0:T0 + LA],
                    in1=e[:, T0 - df:T0 - df + LA], op=ADD)
                nc.vector.tensor_tensor(
                    out=ut[:], in0=t[:, T0:T0 + LA],
                    in1=t[:, T0 - df:T0 - df + LA], op=ADD)
                return ue, ut

            def emit_mms(k, tiles, pe, pt, first, chunks=range(N_CHUNKS),
                         qs="et", pre=None):
                s, e, t, df = tiles
                stop = k == len(CANON) - 1
                if pre is not None:
                    ue, ut = pre
                    views = [(ue, ut, 0)]
                else:
                    views = [(e, t, T0)]
                    if df != 0:
                        views.append((e, t, T0 - df))
                for esrc, tsrc, to in views:
                    for ci in chunks:
                        if "e" in qs:
                            pev = pe[ci // 2][:, (ci % 2) * MM_CHUNK:
                                              (ci % 2 + 1) * MM_CHUNK]
                            mve = view2d(esrc[:], to + ci * RPC * WP,
                                         RPC, W, WP)
                            nc.tensor.matmul(pev, ident[:], mve,
                                             start=first[ci], stop=stop)
                        if "t" in qs:
                            ptv = pt[ci // 2][:, (ci % 2) * MM_CHUNK:
                                              (ci % 2 + 1) * MM_CHUNK]
                            mvt = view2d(tsrc[:], to + ci * RPC * WP,
                                         RPC, W, WP)
                            nc.tensor.matmul(ptv, ident[:], mvt,
                                             start=first[ci], stop=stop)
                        if qs == "et":
                            first[ci] = False

            def emit_one(k, tiles, pe, pt, first, chunks=range(N_CHUNKS)):
                pre = emit_preadd(tiles) if CANON[k] in PREADD else None
                emit_mms(k, tiles, pe, pt, first, chunks, pre=pre)

            def emit_final_half(b, h, pe, pt):
                base = b * NX
                xc = view2d(x[:], base + T0 + h * (BR // 2) * WP,
                            BR // 2, W, WP)
                den = fin.tile([P, HALF], F32, tag="den%d" % h)
                r = fin.tile([P, HALF], F32, tag="r%d" % h)
                out = fin.tile([P, HALF], F32, tag="out%d" % h)
                nc.vector.tensor_tensor(out=den[:], in0=pe[h][:], in1=xc,
                                        op=MULT)
                nc.vector.reciprocal_approx_fast(out=r[:], in_=den[:])
                nc.vector.tensor_tensor(out=out[:], in0=pt[h][:], in1=r[:],
                                        op=MULT)
                nc.sync.dma_start(
                    out=y_out[:, b * LC + h * HALF:b * LC + (h + 1) * HALF],
                    in_=out[:])

            def emit_batch_tail(b, pe, pt, first):
                # all e-matmuls of the last N_TAIL tiles before their
                # t-matmuls: closes the pe accumulation groups early so
                # den/recip run under the trailing t accumulation
                ks = list(range(len(CANON) - N_TAIL, len(CANON)))
                tls = [emit_tile(b, k) for k in ks]
                for k, tl in zip(ks, tls):
                    emit_mms(k, tl, pe, pt, first, qs="e")
                for k, tl in zip(ks, tls):
                    emit_mms(k, tl, pe, pt, first, qs="t")

            pe_prev = pt_prev = None
            n_mid = len(CANON) - N_TAIL
            for b in range(N_BATCH):
                pe = [ps.tile([P, HALF], F32, tag="pe%d" % i,
                              name="pe%d" % i) for i in (0, 1)]
                pt = [ps.tile([P, HALF], F32, tag="pt%d" % i,
                              name="pt%d" % i) for i in (0, 1)]
                first = [True] * N_CHUNKS
                if b == 0:
                    for k in range(n_mid):
                        emit_one(k, emit_tile(b, k), pe, pt, first)
                else:
                    # overlap the previous batch's finals (which release the
                    # PSUM banks) with this batch's first tile computes
                    t0_tiles = emit_tile(b, 0)
                    emit_final_half(b - 1, 0, pe_prev, pt_prev)
                    emit_one(0, t0_tiles, pe, pt, first, chunks=(0, 1))
                    t1_tiles = emit_tile(b, 1)
                    emit_final_half(b - 1, 1, pe_prev, pt_prev)
                    emit_one(0, t0_tiles, pe, pt, first, chunks=(2, 3))
                    emit_one(1, t1_tiles, pe, pt, first)
                    for k in range(2, n_mid):
                        emit_one(k, emit_tile(b, k), pe, pt, first)
                emit_batch_tail(b, pe, pt, first)
                pe_prev, pt_prev = pe, pt
            emit_final_half(N_BATCH - 1, 0, pe_prev, pt_prev)
            emit_final_half(N_BATCH - 1, 1, pe_prev, pt_prev)
    nc.compile()
    return nc


_NC_CACHE = {}


def _get_nc():
    if "nc" not in _NC_CACHE:
        _NC_CACHE["nc"] = build_nc()
    return _NC_CACHE["nc"]


def make_slabs(imgs):
    """[32,128,128] fp32 (one core) -> [128, 2*3808] bf16 slab layout."""
    xb = imgs.astype(NP_BF16)
    xp = np.pad(xb, ((0, 0), (PADV, PADV), (PADH, PADH)))
    rows = (np.arange(RB_N) * BR)[:, None] + np.arange(SLAB)
    out = np.empty((P, N_BATCH, NX), NP_BF16)
    for b in range(N_BATCH):
        part = xp[b * IMG_PER_BATCH:(b + 1) * IMG_PER_BATCH]  # [16,140,136]
        sl = part[:, rows, :]              # [16, 8, 28, 136]
        sl = sl.transpose(1, 0, 2, 3)      # [8, 16, 28, 136] p = rb*16+img
        out[:, b, :] = sl.reshape(P, NX)
    return np.ascontiguousarray(out.reshape(P, N_BATCH * NX))


def unslab_out(y):
    """[128, 2*2048] fp32 -> [32, 128, 128]."""
    res = np.empty((IMG_PER_CORE, H, W), np.float32)
    for b in range(N_BATCH):
        yb = y[:, b * LC:(b + 1) * LC].reshape(RB_N, IMG_PER_BATCH, BR, W)
        res[b * IMG_PER_BATCH:(b + 1) * IMG_PER_BATCH] = (
            yb.transpose(1, 0, 2, 3).reshape(IMG_PER_BATCH, H, W)
        )
    return res


def run(x, **spmd_kwargs):
    nc = _get_nc()
    imgs = np.ascontiguousarray(np.asarray(x).reshape(N_IMG_TOTAL, H, W))
    imgs = imgs.astype(np.float32, copy=False)
    ident = np.eye(P, dtype=NP_BF16)
    in_maps = [
        {"x": make_slabs(imgs[i * IMG_PER_CORE:(i + 1) * IMG_PER_CORE]),
         "ident": ident}
        for i in range(N_CORES)
    ]
    res = run_bass_kernel_spmd(nc, in_maps, core_ids=list(range(N_CORES)),
                               **spmd_kwargs)
    out = np.concatenate(
        [unslab_out(res.results[i]["y"]) for i in range(N_CORES)],
        axis=0,
    )
    return out.reshape(B, C, H, W).astype(np.float32, copy=False), res


def kernel(x):
    out, _ = run(x)
    return out
